# revision 19
# baseline (speedup 1.0000x reference)
"""Trainium2 Bass kernel for nn_BPDRLayer (FreMLP GNN message passing).

Reformulation (validated against the jax reference):
  * FFT / IFFT are linear maps along the feature axis, so the whole FreMLP
    folds into two dense matmuls per edge:
        T   = relu(x_aug @ W1_k)           x_aug = [hidden[src], ea, te, 1]
        msg = T @ W2_k                     (softshrink(relu(z)) == relu(z - lam))
    with W1_k / W2_k precomputed on the host in float64 from the DFT matrices
    and the model weights (fre_w, comb_w even rows, lin_w all folded in).
  * Per-edge band masks are scalars that commute through the matmuls; band
    membership is computed on the host (Parseval: energy = 192*||x||^2, a
    pure function of the inputs with a ~1e5x margin to the thresholds) and
    becomes the edge -> band grouping of the schedule.  Masked-out bands
    contribute constant per-edge vectors, folded into a per-node init table
    (deg/cnt weighted) on the host.  The boundary_condition @ lin_w term is
    also folded into that init table on the host.
  * All device matmuls run in fp8 (e4m3) DoubleRow mode (2 rows/cycle):
    mm1 contracts the 193-dim input as 2 k-planes (hidden | ea/te/bias),
    mm2 contracts the 256-dim T as 2 k-planes, and the one-hot segment-sum
    contracts edge-tile PAIRS as the 2 k-planes.  W2 is pre-scaled by 2^11
    on the host so fp8 msg values sit in e4m3's normal range; the layernorm
    epsilon is scaled by 2^22 to compensate exactly (LN is scale-invariant
    otherwise).  End-to-end rel err ~2e-3 (budget 2e-2).
  * Sharding: edges are sorted by destination and split at node boundaries
    into 8 per-core groups, so each core owns a disjoint node range and no
    inter-core collective is needed.  Within a core, nodes are bin-packed
    into 128-node tiles with a fixed edge budget.  hidden[src] rows are
    pre-gathered on the host into a contiguous feature-major edge stream,
    so the device only ever does dense streaming DMA (no dma_gather).
    PSUM->SBUF drains are spread across the Scalar, Vector and GpSimd
    engines; the layernorm tail (normalize*g+b, relu) is a single fused
    scalar-engine activation with per-partition scale/bias.

kernel(**inputs) takes the full unsharded inputs and returns the full
[50000, 128] float32 output.  Shapes are hardcoded to this problem size.
"""

import heapq
import numpy as np
import ml_dtypes

F8 = ml_dtypes.float8_e4m3

NN = 50000
EMB = 128
EA = 32
ET = 32
DIN = EMB + EA + ET            # 192
NB = 3
LAM = 0.01
LN_EPS = 1e-5
NCORES = 8
P = 128
C2 = 2048.0                    # fp8 msg scale (folded into W2 / binit / eps)

TILE_EDGE_CAP = 768            # edge slots per 128-node tile
NTILES0 = 53                   # starting bin count per core
TPC = 4                        # node tiles per compute chunk

_prog_cache = {}


# --------------------------------------------------------------------------
# host-side weight folding (float64)
# --------------------------------------------------------------------------
def _fold_weights(inp):
    f8 = np.float64
    r1 = np.asarray(inp["r1"], f8)
    i1 = np.asarray(inp["i1"], f8)
    rb1 = np.asarray(inp["rb1"], f8)
    ib1 = np.asarray(inp["ib1"], f8)
    fre_w = np.asarray(inp["fre_w"], f8)
    fre_b = np.asarray(inp["fre_b"], f8)
    comb_w = np.asarray(inp["comb_w"], f8)
    comb_b = np.asarray(inp["comb_b"], f8)
    lin_w = np.asarray(inp["lin_w"], f8)

    n = np.arange(DIN)
    ang = 2.0 * np.pi * np.outer(n, n) / DIN
    Cr, Ci = np.cos(ang), -np.sin(ang)          # xr = x@Cr, xi = x@Ci

    W1 = np.zeros((NB, DIN + 1, 2 * EMB), f8)
    for k in range(NB):
        W1[k, :DIN, :EMB] = Cr @ r1[k] - Ci @ i1[k]
        W1[k, :DIN, EMB:] = Ci @ r1[k] + Cr @ i1[k]
        W1[k, DIN, :EMB] = rb1[k] - LAM
        W1[k, DIN, EMB:] = ib1[k] - LAM

    N2 = NB * EMB
    m = np.arange(N2)
    ang2 = 2.0 * np.pi * np.outer(m, m) / N2
    A = np.cos(ang2) / N2                        # y = Yr@A + Yi@B
    B = -np.sin(ang2) / N2
    Ce = comb_w[0::2]
    G = fre_w @ Ce @ lin_w
    AG, BG = A @ G, B @ G
    W2 = np.zeros((NB, 2 * EMB, EMB), f8)
    d = np.zeros((NB, EMB), f8)
    for k in range(NB):
        W2[k, :EMB] = AG[k * EMB:(k + 1) * EMB]
        W2[k, EMB:] = BG[k * EMB:(k + 1) * EMB]
        cr = np.maximum(rb1[k] - LAM, 0.0)
        ci = np.maximum(ib1[k] - LAM, 0.0)
        d[k] = cr @ W2[k, :EMB] + ci @ W2[k, EMB:]
    bias3 = (fre_b @ Ce + comb_b) @ lin_w
    return W1, W2, d, bias3


# --------------------------------------------------------------------------
# host-side planning
# --------------------------------------------------------------------------
def _bin_pack(nodes, degs, ntiles, edge_cap):
    """Pack (node, deg) into <=ntiles bins of <=128 nodes / <=edge_cap edges.
    Returns list of node-id lists, or None if it doesn't fit."""
    order = np.argsort(-degs, kind="stable")
    heap = [(-edge_cap, t) for t in range(ntiles)]
    heapq.heapify(heap)
    bins = [[] for _ in range(ntiles)]
    rem_e = [edge_cap] * ntiles
    rem_n = [128] * ntiles
    stash = []
    for ni in order:
        d = int(degs[ni])
        placed = False
        while heap:
            nre, t = heapq.heappop(heap)
            if -nre != rem_e[t] or rem_n[t] == 0:
                continue           # stale entry
            if rem_e[t] >= d:
                bins[t].append(int(nodes[ni]))
                rem_e[t] -= d
                rem_n[t] -= 1
                if rem_n[t] > 0:
                    heapq.heappush(heap, (-rem_e[t], t))
                placed = True
                break
            else:
                stash.append((nre, t))
        for it in stash:
            heapq.heappush(heap, it)
        stash.clear()
        if not placed:
            return None
    return bins


def _plan(inp):
    f8 = np.float64
    hidden = np.asarray(inp["hidden"], np.float32)
    ea = np.asarray(inp["edge_attr"], np.float32)
    te = np.asarray(inp["edge_time_emb"], np.float32)
    bc = np.asarray(inp["boundary_condition"], np.float32)
    alpha = np.asarray(inp["alpha"], f8)
    lin_b = np.asarray(inp["lin_b"], f8)
    lin_w32 = np.asarray(inp["lin_w"], np.float32)
    ln_g = np.asarray(inp["ln_g"], np.float32)
    ln_b = np.asarray(inp["ln_b"], np.float32)
    ln_trivial = bool(np.all(ln_g == 1.0) and np.all(ln_b == 0.0))
    eidx = np.asarray(inp["edge_index"]).astype(np.int64)
    src, dst = eidx[0], eidx[1]
    E = src.shape[0]

    W1, W2, d, bias3 = _fold_weights(inp)

    # band membership via Parseval (margin to thresholds is ~1e5x here)
    h2 = DIN * (hidden.astype(f8) ** 2).sum(1)
    e2 = DIN * ((ea.astype(f8) ** 2).sum(1) + (te.astype(f8) ** 2).sum(1))
    energy = h2[src] + e2
    S = energy.sum()
    masks = np.zeros((NB, E), bool)
    for k in range(NB):
        factor = (2.0 * (k + 1) - 1.0) / (2.0 * NB)
        qk = alpha[k] * factor * S
        bk = S / (alpha[k] * 2.0 * NB)
        masks[k] = (energy >= qk - bk) & (energy <= qk + bk)
    bands = [k for k in range(NB) if masks[k].any()]

    # dst-sorted edge partition across cores; split at node boundaries
    order = np.argsort(dst, kind="stable")
    sdst = dst[order]
    bounds = [0]
    for c in range(1, NCORES):
        p0 = (c * E) // NCORES
        while p0 < E and sdst[p0] == sdst[p0 - 1]:
            p0 += 1
        bounds.append(p0)
    bounds.append(E)
    node_lo = [0] + [int(sdst[bounds[c]]) for c in range(1, NCORES)] + [NN]

    deg_all = np.bincount(dst, minlength=NN)
    deg_band = [np.bincount(dst[masks[k]], minlength=NN) for k in bands]

    # bin-pack each core's nodes
    ntiles = NTILES0
    while True:
        packs = []
        ok = True
        for c in range(NCORES):
            nodes = np.arange(node_lo[c], node_lo[c + 1])
            degs = deg_all[nodes]
            b = _bin_pack(nodes, degs, ntiles, TILE_EDGE_CAP)
            if b is None:
                ok = False
                break
            packs.append(b)
        if ok:
            break
        ntiles += 1
        assert ntiles < 96, "bin packing failed"
    act_tiles = ntiles
    while ntiles % TPC:
        ntiles += 1
        packs = [b + [[]] for b in packs]

    # per-band per-tile edge budget S_k (identical across cores/tiles)
    def rup(x, g):
        return -(-x // g) * g

    S_k = []
    for bi, k in enumerate(bands):
        mx = 0
        for c in range(NCORES):
            for tile_nodes in packs[c]:
                if tile_nodes:
                    mx = max(mx, int(deg_band[bi][np.asarray(tile_nodes)].sum()))
        S_k.append(int(rup(max(mx, 128), P)))
    if (sum(S_k) // P) % 2:
        S_k[-1] += P               # segsum pairs edge tiles: need even count
    SEG = sum(S_k)                 # edge slots per node tile
    TOT = ntiles * SEG             # edge stream length per core

    # adjacency: edges grouped by (dst, band) for fast per-node pulls
    eb_sorted = []
    eb_ptr = []
    for bi, k in enumerate(bands):
        ids = np.nonzero(masks[k])[0]
        ids = ids[np.argsort(dst[ids], kind="stable")]
        ptr = np.searchsorted(dst[ids], np.arange(NN + 1))
        eb_sorted.append(ids)
        eb_ptr.append(ptr)

    in_maps = []
    hid8 = hidden.astype(F8)
    bc_lin = bc @ lin_w32                       # boundary fold (host, free)
    # w1 packed for DoubleRow: [band, half, p, kplane, m]
    w1_pk = np.zeros((len(bands), 2, P, 2, P), F8)
    w2_pk = np.zeros((len(bands), P, 2, P), F8)
    for bi, k in enumerate(bands):
        w1f = W1[k].astype(np.float32)          # [193, 256]
        for h in range(2):
            w1_pk[bi, h, :, 0, :] = w1f[:P, h * P:(h + 1) * P].astype(F8)
            w1_pk[bi, h, :DIN - P + 1, 1, :] = \
                w1f[P:, h * P:(h + 1) * P].astype(F8)
        w2s = (W2[k] * C2).astype(np.float32)   # [256, 128] scaled
        w2_pk[bi, :, 0, :] = w2s[:P].astype(F8)
        w2_pk[bi, :, 1, :] = w2s[P:].astype(F8)
    lngb = np.zeros((2, P, EMB), np.float32)
    lngb[0] = np.broadcast_to(ln_g, (P, EMB))
    lngb[1] = np.broadcast_to(ln_b, (P, EMB))
    bias3_32 = bias3.astype(np.float32)
    d32 = d.astype(np.float32)

    gmaps = []
    for c in range(NCORES):
        slot_src = np.zeros(TOT, np.int64)
        er = np.zeros((P, TOT), F8)
        onehot = np.zeros((TOT, P), F8)
        binit_pk = np.zeros((P, ntiles * EMB), np.float16)
        gmap = np.full(ntiles * P, -1, np.int64)

        for t, tile_nodes in enumerate(packs[c]):
            tn = np.asarray(sorted(tile_nodes), np.int64)
            nn_t = len(tn)
            base = t * SEG
            rowbase = t * P
            gmap[rowbase:rowbase + nn_t] = tn
            # init rows: lin_b + bc@lin_w + deg*bias3 + sum_k cnt_inactive_k*d_k
            if nn_t:
                acc = (deg_all[tn].astype(np.float32)[:, None] * bias3_32[None, :]
                       + lin_b.astype(np.float32)[None, :]
                       + bc_lin[tn])
                for bi, k in enumerate(bands):
                    cnt = (deg_all[tn] - deg_band[bi][tn]).astype(np.float32)
                    acc += cnt[:, None] * d32[k][None, :]
                for k in range(NB):
                    if k not in bands:      # band inactive everywhere
                        acc += deg_all[tn].astype(np.float32)[:, None] * d32[k][None, :]
                binit_pk[:nn_t, t * EMB:(t + 1) * EMB] = C2 * acc
            # edge slots, per band segment
            segoff = 0
            for bi, k in enumerate(bands):
                ids_parts = []
                rows_parts = []
                for j in range(nn_t):
                    nid = tn[j]
                    lo_p, hi_p = eb_ptr[bi][nid], eb_ptr[bi][nid + 1]
                    if hi_p > lo_p:
                        eids = eb_sorted[bi][lo_p:hi_p]
                        ids_parts.append(eids)
                        rows_parts.append(np.full(hi_p - lo_p, j, np.int64))
                if ids_parts:
                    eids = np.concatenate(ids_parts)
                    rows = np.concatenate(rows_parts)
                    n = len(eids)
                    assert n <= S_k[bi]
                    sl = slice(base + segoff, base + segoff + n)
                    slot_src[sl] = src[eids]
                    er[:EA, sl] = ea[eids].T.astype(F8)
                    er[EA:EA + ET, sl] = te[eids].T.astype(F8)
                    er[DIN - P, sl] = F8(1.0)
                    onehot[np.arange(base + segoff, base + segoff + n), rows] = F8(1.0)
                segoff += S_k[bi]

        xh = np.ascontiguousarray(hid8[slot_src].T)         # [128, TOT] fp8
        oh3 = onehot.reshape(TOT // P, P, P).transpose(1, 0, 2).copy()
        im = {
            "xh": xh,
            "er": er,
            "onehot": oh3,
            "w1": w1_pk,
            "w2": w2_pk,
            "binit": binit_pk,
            "ident": np.eye(P, dtype=np.float16),
        }
        if not ln_trivial:
            im["lngb"] = lngb
        in_maps.append(im)
        gmaps.append(gmap)

    # band index of each 128-edge tile within a SEG (for weight selection)
    seg_band = []
    for bi in range(len(bands)):
        seg_band += [bi] * (S_k[bi] // P)
    while len(seg_band) < SEG // P:
        seg_band.append(len(bands) - 1)        # parity pad tile

    sig = (ntiles, act_tiles, tuple(S_k), len(bands), ln_trivial)
    meta = {"ntiles": ntiles, "act_tiles": act_tiles,
            "S_k": S_k, "SEG": SEG, "TOT": TOT,
            "nbands": len(bands), "seg_band": seg_band, "gmaps": gmaps,
            "node_lo": node_lo, "ln_trivial": ln_trivial}
    return sig, meta, in_maps


# --------------------------------------------------------------------------
# device program
# --------------------------------------------------------------------------
def _build_program(meta):
    import concourse.bacc as bacc
    import concourse.tile as tile
    from concourse import mybir

    ntiles = meta["ntiles"]
    ACT = meta["act_tiles"]
    SEG = meta["SEG"]
    TOT = meta["TOT"]
    nbands = meta["nbands"]
    seg_band = meta["seg_band"]
    S_k = meta["S_k"]
    ln_trivial = meta["ln_trivial"]
    f8d = mybir.dt.float8e4
    f16 = mybir.dt.float16
    f32 = mybir.dt.float32
    AF = mybir.ActivationFunctionType
    ALU = mybir.AluOpType
    DR = mybir.MatmulPerfMode.DoubleRow

    assert ntiles % TPC == 0
    CH = TPC * SEG               # edges per compute chunk
    SEGB = -(-SEG // 512) * 512  # bank-aligned psum width per node tile
    NCHUNK = ntiles // TPC
    NET = CH // P                # edge tiles per chunk

    nc = bacc.Bacc("TRN2", target_bir_lowering=False, debug=False,
                   enable_asserts=False, num_devices=NCORES)

    xh_d = nc.dram_tensor("xh", [P, TOT], f8d, kind="ExternalInput")
    er_d = nc.dram_tensor("er", [P, TOT], f8d, kind="ExternalInput")
    oh_d = nc.dram_tensor("onehot", [P, TOT // P, P], f8d, kind="ExternalInput")
    w1_d = nc.dram_tensor("w1", [nbands, 2, P, 2, P], f8d, kind="ExternalInput")
    w2_d = nc.dram_tensor("w2", [nbands, P, 2, P], f8d, kind="ExternalInput")
    binit_d = nc.dram_tensor("binit", [P, ntiles * EMB], f16, kind="ExternalInput")
    ident_d = nc.dram_tensor("ident", [P, P], f16, kind="ExternalInput")
    if not ln_trivial:
        lngb_d = nc.dram_tensor("lngb", [2, P, EMB], f32, kind="ExternalInput")
    out_d = nc.dram_tensor("out", [P, ntiles * EMB], f16, kind="ExternalOutput")

    with tile.TileContext(nc) as tc:
        with (
            tc.tile_pool(name="singles", bufs=1) as singles,
            tc.tile_pool(name="edges", bufs=3) as epool,
            tc.tile_pool(name="nodes", bufs=3) as npool,
            tc.tile_pool(name="psumT", bufs=2, space="PSUM") as psumT,
            tc.tile_pool(name="psumM", bufs=2, space="PSUM") as psumM,
            tc.tile_pool(name="psumN", bufs=2, space="PSUM") as psumN,
        ):
            # ---- constants ----
            w1_sb, w2_sb = [], []
            for bi in range(nbands):
                h0 = singles.tile([P, 2, P], f8d, tag=f"w1h0{bi}")
                h1 = singles.tile([P, 2, P], f8d, tag=f"w1h1{bi}")
                nc.sync.dma_start(out=h0[:], in_=w1_d[bi, 0])
                nc.sync.dma_start(out=h1[:], in_=w1_d[bi, 1])
                w2t = singles.tile([P, 2, P], f8d, tag=f"w2{bi}")
                nc.sync.dma_start(out=w2t[:], in_=w2_d[bi])
                w1_sb.append((h0, h1))
                w2_sb.append(w2t)
            if not ln_trivial:
                lng_sb = singles.tile([P, EMB], f32)
                lnb_sb = singles.tile([P, EMB], f32)
                nc.sync.dma_start(out=lng_sb[:], in_=lngb_d[0])
                nc.sync.dma_start(out=lnb_sb[:], in_=lngb_d[1])
            eps_sb = singles.tile([P, 1], f32)
            nc.vector.memset(eps_sb[:], LN_EPS * C2 * C2)
            ident_sb = singles.tile([P, P], f16, tag="ident")
            nc.sync.dma_start(out=ident_sb[:], in_=ident_d[:])
            # warm the activation table while the first chunk streams in
            warm = singles.tile([P, 1], f32, tag="warm")
            nc.scalar.activation(out=warm[:], in_=eps_sb[:], func=AF.Relu)
            nc.scalar.activation(out=warm[:], in_=eps_sb[:], func=AF.Sqrt,
                                 bias=eps_sb[:], scale=1.0)

            # psum->sbuf drains are split across scalar and vector only
            # (GPSIMD cannot access PSUM on hardware)
            t_eng = ["scalar", "vector", "scalar", "vector",
                     "scalar", "vector", "scalar", "scalar"]
            m_eng = ["vector", "scalar", "vector", "vector", "scalar", "vector"]

            # LayerNorm tail for chunk `prev`, software-pipelined one chunk
            # behind the matmul stream so its cross-engine deps never stall
            # the scalar/vector instruction queues at a chunk boundary.
            def ln_stats(prev_ci, pn, att):
                mva = npool.tile([P, TPC, 2], f32, tag="mva")
                for tt in range(att):
                    stats = npool.tile([P, 6], f32, tag=f"st{tt}")
                    nc.vector.bn_stats(out=stats[:], in_=pn[:, tt, :])
                    nc.vector.bn_aggr(out=mva[:, tt, :], in_=stats[:])
                return mva

            def ln_tail(prev_ci, pn, mva, att):
                sq = npool.tile([P, TPC], f32, tag="sq")
                nc.scalar.activation(out=sq[:, :att], in_=mva[:, :att, 1],
                                     func=AF.Sqrt, bias=eps_sb[:],
                                     scale=1.0)
                rs = npool.tile([P, TPC], f32, tag="rs")
                nc.vector.reciprocal(out=rs[:, :att], in_=sq[:, :att])
                nmu = npool.tile([P, TPC], f32, tag="nmu")
                nc.vector.scalar_tensor_tensor(out=nmu[:, :att],
                                               in0=mva[:, :att, 0],
                                               scalar=-1.0, in1=rs[:, :att],
                                               op0=ALU.mult, op1=ALU.mult)
                ot = npool.tile([P, TPC * EMB], f16, tag="ot")
                for tt in range(att):
                    if ln_trivial:
                        # relu((acc-mu)*rs) in one fused scalar op
                        nc.scalar.activation(
                            out=ot[:, tt * EMB:(tt + 1) * EMB],
                            in_=pn[:, tt, :],
                            func=AF.Relu, bias=nmu[:, tt:tt + 1],
                            scale=rs[:, tt:tt + 1])
                    else:
                        nt = npool.tile([P, EMB], f32, tag=f"nt{tt}")
                        nc.scalar.activation(out=nt[:], in_=pn[:, tt, :],
                                             func=AF.Copy, bias=0.0,
                                             scale=rs[:, tt:tt + 1])
                        nc.vector.tensor_scalar(out=nt[:], in0=nt[:],
                                                scalar1=nmu[:, tt:tt + 1],
                                                scalar2=None, op0=ALU.add)
                        nc.vector.tensor_mul(out=nt[:], in0=nt[:],
                                             in1=lng_sb[:])
                        nc.vector.tensor_add(out=nt[:], in0=nt[:],
                                             in1=lnb_sb[:])
                        nc.scalar.activation(
                            out=ot[:, tt * EMB:(tt + 1) * EMB], in_=nt[:],
                            func=AF.Relu)
                nc.sync.dma_start(
                    out=out_d[:, prev_ci * TPC * EMB:
                              prev_ci * TPC * EMB + att * EMB],
                    in_=ot[:, :att * EMB])

            prev = None
            for ci in range(NCHUNK):
                att = min(TPC, ACT - ci * TPC)   # active node tiles
                off = ci * CH            # global edge position offset
                xe = epool.tile([P, 2, CH], f8d, tag="xe")
                oh_sb = epool.tile([P, NET, P], f8d, tag="oh")
                if ci == 0:
                    # split the first chunk's streams per node tile so the
                    # pipeline starts as soon as tile 0's slice lands
                    for tt in range(TPC):
                        s0, s1 = tt * SEG, (tt + 1) * SEG
                        nc.sync.dma_start(out=xe[:, 0, s0:s1],
                                          in_=xh_d[:, off + s0:off + s1])
                        nc.sync.dma_start(out=xe[:, 1, s0:s1],
                                          in_=er_d[:, off + s0:off + s1])
                        nc.sync.dma_start(
                            out=oh_sb[:, s0 // P:s1 // P, :],
                            in_=oh_d[:, (off + s0) // P:(off + s1) // P, :])
                else:
                    nc.sync.dma_start(out=xe[:, 0, :],
                                      in_=xh_d[:, off:off + CH])
                    nc.sync.dma_start(out=xe[:, 1, :],
                                      in_=er_d[:, off:off + CH])
                    nc.sync.dma_start(
                        out=oh_sb[:],
                        in_=oh_d[:, off // P:(off + CH) // P, :])
                bi_t = npool.tile([P, TPC * EMB], f16, tag="binit")
                nc.sync.dma_start(
                    out=bi_t[:],
                    in_=binit_d[:, ci * TPC * EMB:(ci + 1) * TPC * EMB])

                if prev is not None:
                    mva_p = ln_stats(*prev)

                # seed the PSUM accumulator with binit via an identity
                # matmul; the one-hot matmuls then accumulate on top
                pn = psumN.tile([P, TPC, EMB], f32, tag="pn")
                nc.tensor.matmul(pn[:].rearrange("p a b -> p (a b)"),
                                 ident_sb[:], bi_t[:],
                                 start=True, stop=False)

                # ---- mm1: T = relu(x_aug @ W1), DoubleRow fp8 ----
                Tsb = epool.tile([P, 2, CH], f8d, tag="T")
                di = 0
                for tt in range(att):
                    segoff = tt * SEG
                    for h in range(2):
                        pt = psumT.tile([P, SEGB], f32, tag="pt")
                        sb0a = 0
                        for bi in range(nbands):
                            sk = S_k[bi]
                            sb0 = sum(S_k[:bi])
                            nb_n = -(-sk // 512)
                            for j in range(nb_n):
                                p0 = sb0a + j * 512
                                p1 = p0 + min(sk - j * 512, 512)
                                n0 = segoff + sb0 + j * 512
                                n1 = n0 + (p1 - p0)
                                nc.tensor.matmul(
                                    pt[:, p0:p1], w1_sb[bi][h][:],
                                    xe[:, :, n0:n1],
                                    start=True, stop=True, perf_mode=DR)
                            sb0a += -(-sk // 512) * 512
                        dst = Tsb[:, h, segoff:segoff + SEG]
                        eng = t_eng[di % len(t_eng)]
                        di += 1
                        if eng == "scalar":
                            nc.scalar.activation(out=dst, in_=pt[:, :SEG],
                                                 func=AF.Relu)
                        else:
                            nc.vector.tensor_scalar_max(out=dst,
                                                        in0=pt[:, :SEG],
                                                        scalar1=0.0)

                if prev is not None:
                    ln_tail(*prev[:2], mva_p, prev[2])

                # ---- mm2: msg = T^T @ W2 (edge-major), DoubleRow fp8 ----
                # groups of 3 edge tiles align drain boundaries with node
                # tiles (SEG//P = 6), so the segment sum never waits on a
                # half-drained group
                msg = epool.tile([P, NET, P], f8d, tag="msg")
                mi = 0
                MG = 3
                net_a = att * (SEG // P)
                for ebase in range(0, net_a, MG):
                    g = min(MG, net_a - ebase)
                    pm = psumM.tile([P, MG * P], f32, tag="pm")
                    for e4 in range(g):
                        et = ebase + e4
                        bi = seg_band[(et * P % SEG) // P]
                        esl = slice(et * P, (et + 1) * P)
                        nc.tensor.matmul(pm[:, e4 * P:(e4 + 1) * P],
                                         Tsb[:, :, esl], w2_sb[bi][:],
                                         start=True, stop=True, perf_mode=DR)
                    dst = msg[:, ebase:ebase + g, :]
                    src_ap = pm[:, :g * P].rearrange("p (a b) -> p a b", a=g)
                    eng = m_eng[mi % len(m_eng)]
                    mi += 1
                    if eng == "scalar":
                        nc.scalar.activation(out=dst, in_=src_ap, func=AF.Copy)
                    else:
                        nc.vector.tensor_copy(out=dst, in_=src_ap)

                # ---- per node tile: paired one-hot segment sum (DoubleRow)
                #      accumulating onto the binit seed ----
                npair = SEG // P // 2
                for tt in range(att):
                    e0 = tt * SEG // P
                    for j in range(npair):
                        psl = slice(e0 + 2 * j, e0 + 2 * j + 2)
                        last = (tt == att - 1) and (j == npair - 1)
                        nc.tensor.matmul(pn[:, tt, :], oh_sb[:, psl, :],
                                         msg[:, psl, :],
                                         start=False, stop=last,
                                         perf_mode=DR,
                                         skip_group_check=not last)
                prev = (ci, pn, att)

            mva_p = ln_stats(*prev)
            ln_tail(*prev[:2], mva_p, prev[2])

    nc.compile()
    return nc


# --------------------------------------------------------------------------
# entry point
# --------------------------------------------------------------------------
def kernel(**inputs):
    from concourse.bass_utils import run_bass_kernel_spmd

    sig, meta, in_maps = _plan(inputs)
    if sig not in _prog_cache:
        _prog_cache[sig] = _build_program(meta)
    nc = _prog_cache[sig]

    res = run_bass_kernel_spmd(nc, in_maps, core_ids=list(range(NCORES)))
    ntiles = meta["ntiles"]
    out = np.zeros((NN, EMB), np.float32)
    for c in range(NCORES):
        gmap = meta["gmaps"][c]
        # out_pk [128, ntiles*EMB] -> [ntiles*128, EMB]
        pk = np.asarray(res.results[c]["out"], np.float32)
        core_out = pk.reshape(P, ntiles, EMB).transpose(1, 0, 2).reshape(-1, EMB)
        valid = gmap >= 0
        out[gmap[valid]] = core_out[valid]
    return out
